# revision 33
# baseline (speedup 1.0000x reference)
"""Fake-attention kernel for trn2: 8 NeuronCores, one batch element per core.

Per core (batch b): out = softmax(k @ q^T) @ v with k/q/v = x @ W.T + b.

Key algebraic reduction: softmax rows are shift-invariant, and
  k_n . q_m = (Wq^T k_n) . x_m + (k_n . bq)
where the second term is constant along the softmax axis m.  So with
  kpp = (Wq^T Wk) x + Wq^T bk      (ONE projection instead of k and q)
softmax(k q^T) == softmax(kpp x^T) exactly.  The q-side of the scores
matmul is the raw (already-resident) xT.

Layout: transposed so the softmax reduction lands on PE partitions and
PV contracts on partitions:
  xT   [f, n]   (host-transposed input)
  kppT [d, n] = Ck^T @ xT + g      (fp32r, exact)
  v    [m, d] = x @ Wv^T           (bf16 inputs, fp32 psum, stored f32r)
  scoresT chunk [m=128, n=512] = xT-chunk as lhsT, kppT-slice as rhs (fp32r)
  p = exp(scoresT), ONE 1536-wide ACT op per 3 chunks (bf16 out)
  outT [d, n-sec] += v-chunk as lhsT, p-chunk as rhs  (PSUM accumulation)

ACT (the only exp engine) is the bottleneck: 131072 exp columns at
0.833ns/col + ~185ns/instruction.  Everything else is arranged to keep
ACT saturated: 86 exp ops of width 1536; PSUM = 2x[128,1536] score
tiles (double buffer) + 2x[128,512] persistent PV accumulators (even/odd
sections); denominators off ACT (DVE bf16 pair-trees at 2x + Pool fp32
chains); v/kpp setup matmuls slotted into PV-accumulator idle windows;
finalize transposes reuse the retired PV psum tile.
"""
import numpy as np

B = 8
N = 4096
D = 128
SEC = 512            # n-section width (PV accumulator width)
NSEC = N // SEC      # 8
NCH = 32             # m-chunks of 128 per section
NG = NSEC * NCH      # 256 chunk units
TCH = 3              # chunks per bulk exp tile
NT = 88              # tiles: 1 + 84*3 + 3*1 = 256 chunks

_cache = {}

# wp pack layout (columns)
_CK = slice(0, 128)        # Ck = Wk^T Wq  (lhsT for kpp projection)
_G = slice(128, 129)       # g = Wq^T bk
_ONES = slice(129, 130)    # 1.0 column
_WVT = slice(130, 258)     # Wv^T
_BVB = slice(258, 386)     # bv broadcast [128, 128]
_ID = slice(386, 514)      # identity (PE transpose)
WPW = 514


def make_wp(Wk, Wq, Wv, bk, bq, bv):
    wp = np.zeros((128, WPW), np.float32)
    wp[:, _CK] = Wk.T @ Wq
    wp[:, _G] = (Wq.T @ bk)[:, None]
    wp[:, _ONES] = 1.0
    wp[:, _WVT] = Wv.T
    wp[:, _BVB] = np.broadcast_to(bv[None, :], (128, 128))
    wp[:, _ID] = np.eye(128, dtype=np.float32)
    return wp


def _build(pool_mod=4, ptp_bufs=6, tree_bufs=14,
           fin_a_at=3, fin_b_at=5):
    import concourse.bass as bass  # noqa
    import concourse.mybir as mybir
    import concourse.tile as tile
    from concourse import bacc

    F32 = mybir.dt.float32
    F32R = mybir.dt.float32r
    BF16 = mybir.dt.bfloat16
    Exp = mybir.ActivationFunctionType.Exp
    ADD = mybir.AluOpType.add
    MUL = mybir.AluOpType.mult

    nc = bacc.Bacc()
    xt = nc.declare_dram_parameter("xt", [D, N], F32R, isOutput=False)
    wp = nc.declare_dram_parameter("wp", [128, WPW], F32R, isOutput=False)
    y = nc.declare_dram_parameter("y", [N, D], F32, isOutput=True)

    xt_dram = xt.rearrange("p (c l) -> p c l", l=128)
    y_dram = y.rearrange("(c p) d -> p c d", p=128)

    with tile.TileContext(nc) as tc:
        with (
            tc.tile_pool(name="big", bufs=1) as big,
            tc.tile_pool(name="ptp", bufs=ptp_bufs) as ptp,
            tc.tile_pool(name="tree", bufs=tree_bufs) as tree,
            tc.tile_pool(name="dop", bufs=5) as dop,
            tc.tile_pool(name="wrk", bufs=4) as wrk,
            tc.tile_pool(name="ts", bufs=2, space="PSUM") as tpool,
            tc.tile_pool(name="pva", bufs=1, space="PSUM") as pvpa,
            tc.tile_pool(name="pvb", bufs=1, space="PSUM") as pvpb,
        ):
            # ---------------- startup DMAs (criticality order) ----------
            wp_sb = big.tile([128, WPW], F32R, tag="wp")
            xg = [None] * 4       # xT group tiles [128, 8, 128] f32
            xg0a = big.tile([128, 4, 128], F32R, tag="xT0a")
            xg0b = big.tile([128, 4, 128], F32R, tag="xT0b")
            nc.sync.dma_start(xg0a[:, 0:2, :], xt_dram[:, 0:2, :])
            nc.scalar.dma_start(wp_sb[:, 0:130], wp[:, 0:130])
            nc.sync.dma_start(xg0a[:, 2:4, :], xt_dram[:, 2:4, :])
            nc.sync.dma_start(xg0b[:], xt_dram[:, 4:8, :])
            nc.scalar.dma_start(wp_sb[:, 130:WPW], wp[:, 130:WPW])
            for gi in range(1, 4):
                t = big.tile([128, 8, 128], F32R, tag=f"xT{gi}",
                             name=f"xT{gi}")
                nc.sync.dma_start(t[:], xt_dram[:, gi * 8:(gi + 1) * 8, :])
                xg[gi] = t

            ckT = wp_sb[:, _CK]
            gcol = wp_sb[:, _G].bitcast(F32)
            ident = wp_sb[:, _ID]
            bv_bc = wp_sb[:, _BVB].bitcast(F32)

            def xchunk(mc):
                """[128, 128] f32r slab of xT for m-chunk mc."""
                if mc < 4:
                    return xg0a[:, mc, :]
                if mc < 8:
                    return xg0b[:, mc - 4, :]
                return xg[mc // 8][:, mc % 8, :]

            def xslab(gi, half):
                """[128, 512] f32r slab (half of group gi)."""
                if gi == 0:
                    t = xg0a if half == 0 else xg0b
                    return t.rearrange("p c f -> p (c f)")
                return xg[gi].rearrange("p c f -> p (c f)")[
                    :, half * 512:(half + 1) * 512]

            # ---------------- PE warmup (clock ramp) --------------------
            wu = big.tile([128, 128], F32, tag="warm")
            nc.vector.memset(wu[:], 1.0)
            tsA = tpool.tile([128, 1536], F32, tag="ts")

            def warm(n=1):
                for _ in range(n):
                    nc.tensor.matmul(tsA[:, 0:64], wu[:], wu[:, 0:64],
                                     start=True, stop=True,
                                     skip_group_check=True)
            warm(2)

            # bf16 helper tensors
            wv_bf = big.tile([128, 128], BF16, tag="wvbf")
            ones_bf = big.tile([128, 1], BF16, tag="onesbf")

            # kppT per-section tiles (separate tiles so scores only wait
            # on their own section's bias write); section 0 split in half
            # for the fastest possible first exp
            kpp0a = big.tile([128, 256], F32R, tag="kpp0a")
            kpp0b = big.tile([128, 256], F32R, tag="kpp0b")
            kpps = [big.tile([128, 512], F32R, tag=f"kpp{s}",
                             name=f"kpp{s}") for s in range(1, 8)]
            # v groups [128, 8, 128] bf16 (PV pairs with bf16 p)
            v_g = [big.tile([128, 8, 128], BF16, tag=f"v{gi}",
                            name=f"v{gi}") for gi in range(4)]
            # bf16 copies of x groups (v projection lhsT)
            xbf = [big.tile([128, 8, 128], BF16, tag=f"xbf{gi}",
                            name=f"xbf{gi}") for gi in range(4)]

            def v_chunk(mc):
                return v_g[mc // 8][:, mc % 8, :]

            def emit_kpp(s, ps, off):
                """kpp projection for section s into ps[:, off:off+512]."""
                nc.tensor.matmul(ps[:, off:off + 512], ckT,
                                 xslab(s // 2, s % 2), start=True, stop=True)
                nc.vector.tensor_scalar_add(kpps[s - 1][:],
                                            ps[:, off:off + 512], gcol)

            def emit_vhalf(h, ps, off, copy_eng):
                """v chunks 4h..4h+3 into ps[:, off:off+512], copy to v_g."""
                for j in range(4):
                    mc = 4 * h + j
                    nc.tensor.matmul(
                        ps[:, off + j * 128:off + (j + 1) * 128],
                        xbf[mc // 8][:, mc % 8, :], wv_bf[:],
                        start=True, stop=True)
                dst = v_g[h // 2][:, (h % 2) * 4:(h % 2) * 4 + 4, :]
                src = ps[:, off:off + 512].rearrange("p (c f) -> p c f", f=128)
                nc.vector.tensor_copy(dst, src)

            # ---------------- startup compute ---------------------------
            nc.gpsimd.tensor_copy(wv_bf[:], wp_sb[:, _WVT])
            nc.gpsimd.tensor_copy(ones_bf[:], wp_sb[:, _ONES])
            nc.gpsimd.tensor_copy(xbf[0][:, 0:4, :], xg0a[:])
            warm(8)
            # startup tile A: warm region | kpp sec0 (split halves so the
            # first scores chunk starts as early as possible)
            nc.tensor.matmul(tsA[:, 512:768], ckT, xslab(0, 0)[:, 0:256],
                             start=True, stop=True)
            nc.vector.tensor_scalar_add(kpp0a[:], tsA[:, 512:768], gcol)
            warm(2)
            nc.tensor.matmul(tsA[:, 1024:1280], ckT,
                             xslab(0, 0)[:, 256:512],
                             start=True, stop=True)
            nc.vector.tensor_scalar_add(kpp0b[:], tsA[:, 1024:1280], gcol)
            warm(2)
            nc.gpsimd.tensor_copy(xbf[0][:, 4:8, :], xg0b[:])

            # persistent PV accumulators: even sections -> accA, odd -> accB
            accA = pvpa.tile([128, 512], F32, tag="pvA")
            accB = pvpb.tile([128, 512], F32, tag="pvB")

            def pv_acc(s):
                return accA if s % 2 == 0 else accB

            # deferred work queue: (due_tile, fn), flushed in due order
            pending = []

            def flush_due(t):
                i = 0
                while i < len(pending):
                    due, fn = pending[i]
                    if due <= t:
                        fn()
                        pending.pop(i)
                    else:
                        i += 1

            # denominator state per live section
            dstate = {}

            def new_dstate(s):
                dstate[s] = {"partials": [], "odd": None}

            new_dstate(0)

            def emit_chain(g, p_slice):
                """Route chunk g's denominator contribution.  The last
                section routes Pool chunks early (mc<16) and pre-collapses
                the tree at mc==29 so the post-exp tail is short."""
                s = g // NCH
                mc = g % NCH
                st = dstate[s]
                last = (s == NSEC - 1)
                is_pool = ((mc % 2 == 1 and mc < 16) if last
                           else mc % pool_mod == pool_mod - 1)
                if is_pool:
                    if st["odd"] is None:
                        st["odd"] = dop.tile([128, 512], F32, tag="dodd", name="dodd")
                        nc.gpsimd.tensor_copy(st["odd"][:], p_slice)
                    else:
                        nc.gpsimd.tensor_tensor(st["odd"][:], st["odd"][:],
                                                p_slice, ADD)
                    return
                if last:
                    # two independent sequential bf16 chains: each add is
                    # gated only by its p slice and the chain's previous add
                    # (~a tile apart), so nothing piles up at the end.  The
                    # final chunks land on cB; the only post-final-exp ops
                    # are cB's last add and the cA+cB fold in fin_a.
                    key = "cB" if (mc % 2 == 1 or mc >= 30) else "cA"
                    chain = st.get(key)
                    if chain is None:
                        t2 = dop.tile([128, 512], BF16, tag=key, name=key)
                        nc.vector.tensor_copy(t2[:], p_slice)
                        st[key] = t2
                    else:
                        nc.vector.tensor_tensor(chain[:], chain[:],
                                                p_slice, ADD)
                    if mc == 17:
                        # Pool's fp32 chain (mc<16) is complete; fold into cA
                        nc.vector.tensor_tensor(st["cA"][:], st["cA"][:],
                                                st["odd"][:], ADD)
                        st["odd"] = None
                    return
                # DVE bf16 binary-counter tree (2x mode: all-bf16 SBUF)
                parts = st["partials"]
                parts.append((p_slice, 0))
                while len(parts) >= 2 and parts[-1][1] == parts[-2][1]:
                    a, lv = parts.pop()
                    b, _ = parts.pop()
                    t2 = tree.tile([128, 512], BF16, tag="dt")
                    nc.vector.tensor_tensor(t2[:], a, b, ADD)
                    parts.append((t2[:], lv + 1))

            def emit_fin_a(s):
                """Close section s: merge denominators, partition-sum matmuls
                into the retired PV tile, reciprocal, o_copy."""
                st = dstate.pop(s)
                acc = pv_acc(s)
                if "cA" in st:
                    dfin_t = tree.tile([128, 512], BF16, tag="dfin")
                    nc.vector.tensor_tensor(dfin_t[:], st["cA"][:],
                                            st["cB"][:], ADD)
                    st["partials"] = [(dfin_t[:], 0)]
                    st["odd"] = None
                parts = st["partials"]
                assert parts, f"empty denominator state for section {s}"
                while len(parts) > 1:
                    a, _ = parts.pop()
                    b, lv = parts.pop()
                    t2 = tree.tile([128, 512], BF16, tag="dt")
                    nc.vector.tensor_tensor(t2[:], a, b, ADD)
                    parts.append((t2[:], lv + 1))
                if st["odd"] is not None:
                    dfin = tree.tile([128, 512], BF16, tag="dfin")
                    nc.vector.tensor_tensor(dfin[:], parts[0][0],
                                            st["odd"][:], ADD)
                    dfin = dfin[:]
                else:
                    dfin = parts[0][0]
                o_copy = wrk.tile([128, 512], F32R, tag="oc")
                if s == NSEC - 1:
                    nc.scalar.copy(o_copy[:], acc[:])
                else:
                    nc.vector.tensor_copy(o_copy[:], acc[:])
                # last section: tpd into an idle T-pool tile (survives the
                # transposes, which reuse acc) and divide directly -- no
                # reciprocal on the tail critical path
                if s == NSEC - 1:
                    tpd = tpool.tile([128, 1536], F32, tag="ts")
                else:
                    tpd = acc
                for b_ in range(4):
                    nc.tensor.matmul(
                        tpd[:, b_:b_ + 1],
                        dfin[:, b_ * 128:(b_ + 1) * 128], ones_bf[:],
                        start=True, stop=True, skip_group_check=True)
                recip = wrk.tile([128, 4], F32, tag="rc")
                nc.vector.reciprocal(recip[:], tpd[:, 0:4])
                return {"o_copy": o_copy, "recip": recip, "s": s}

            def emit_fin_b(fs):
                """Transposes + scale + bias + DMA out for section fs['s']."""
                s = fs["s"]
                acc = pv_acc(s)
                o_copy = fs["o_copy"]
                for b_ in range(4):
                    nc.tensor.transpose(
                        acc[:, b_ * 128:(b_ + 1) * 128].bitcast(F32R),
                        o_copy[:, b_ * 128:(b_ + 1) * 128],
                        ident)
                out_g = wrk.tile([128, 4, 128], F32, tag="og")
                tpo_v = acc[:, 0:512].rearrange("p (b l) -> p b l", b=4)
                recip = fs["recip"]
                for b_ in range(4):
                    nc.vector.scalar_tensor_tensor(
                        out_g[:, b_, :], tpo_v[:, b_, :], recip[:, b_:b_ + 1],
                        bv_bc, MUL, ADD)
                    if b_ == 1:
                        nc.sync.dma_start(
                            y_dram[:, s * 4:s * 4 + 2, :], out_g[:, 0:2, :])
                if s == NSEC - 1:
                    nc.scalar.dma_start(
                        y_dram[:, s * 4 + 2:s * 4 + 4, :], out_g[:, 2:4, :])
                else:
                    nc.sync.dma_start(
                        y_dram[:, s * 4 + 2:s * 4 + 4, :], out_g[:, 2:4, :])

            def emit_pv_and_chain(p_tile, chunks):
                for j, g in enumerate(chunks):
                    s = g // NCH
                    mc = g % NCH
                    if mc == 0 and s not in dstate:
                        new_dstate(s)
                    psl = p_tile[:, j * 512:(j + 1) * 512]
                    nc.tensor.matmul(
                        pv_acc(s)[:], v_chunk(mc), psl,
                        start=(mc == 0), stop=(mc == NCH - 1),
                        skip_group_check=True)
                    emit_chain(g, psl)
                    if mc == NCH - 1:
                        fs_box = {}

                        def fa(fs_box=fs_box, sv=s):
                            fs_box["fs"] = emit_fin_a(sv)

                        def fb(fs_box=fs_box):
                            emit_fin_b(fs_box["fs"])
                        tcur = tile_of(g)
                        pending.append((tcur + fin_a_at, fa))
                        pending.append((tcur + fin_b_at, fb))

            # staged setup: kpp1 + v halves 0..7 ride accB before its first
            # PV use (section 1 starts ~tile 11); kpp 2..7 ride accA idle
            # windows right after each even section's finalize.
            def setup_step(step):
                def run():
                    if step == -1:
                        emit_kpp(1, accB, 0)
                    elif 0 <= step < 8:
                        emit_vhalf(step, accB, 0, "gp")
                    elif step == 8:
                        emit_kpp(2, accA, 0)
                        emit_kpp(3, accA, 0)
                    elif step == 9:
                        emit_kpp(4, accA, 0)
                        emit_kpp(5, accA, 0)
                    elif step == 10:
                        emit_kpp(6, accA, 0)
                        emit_kpp(7, accA, 0)
                return run

            pending.append((0, setup_step(-1)))
            for step in range(8):
                pending.append((step, setup_step(step)))

            def xbf_copy(gi):
                def run():
                    nc.vector.tensor_copy(xbf[gi][:], xg[gi][:])
                return run
            pending.append((0, xbf_copy(1)))
            pending.append((2, xbf_copy(2)))
            pending.append((5, xbf_copy(3)))
            # accA windows: after fin_b(0) ~tile 10+fin_b_at, after
            # fin_b(2) ~tile 31+fin_b_at, after fin_b(4) ~tile 53+fin_b_at.
            pending.append((11 + fin_b_at + 1, setup_step(8)))
            pending.append((33 + fin_b_at + 1, setup_step(9)))
            pending.append((54 + fin_b_at + 1, setup_step(10)))

            # ---------------- main stream -------------------------------
            # Emission order per iteration t:
            #   exp(t) [ACT] ; scores(t+1) [PE, gated on exp(t-1) via the
            #   T-buffer rotation -- runs immediately when exp(t-1) ends] ;
            #   deferred setup/finalize ; PV+chain(t-1) [gated on exp(t-1)].
            # This keeps scores(t+1) AHEAD of PV(t-1) on the in-order PE
            # stream so exp(t+1) is never starved.
            def tile_chunks(t):
                # tile 0: single chunk (fast first exp); tiles 1..84: three
                # chunks; tiles 85..87: single chunks (short post-exp tail).
                if t == 0:
                    return [0]
                if t <= 84:
                    return list(range(3 * t - 2, 3 * t + 1))
                return [252 + (t - 84)]

            def tile_of(g):
                if g == 0:
                    return 0
                if g <= 252:
                    return (g + 2) // 3
                return 84 + (g - 252)

            def emit_scores(t):
                chunks = tile_chunks(t)
                ts = tpool.tile([128, 1536], F32, tag="ts")
                for j, g in enumerate(chunks):
                    s = g // NCH
                    if s == 0:
                        nc.tensor.matmul(ts[:, j * 512:j * 512 + 256],
                                         xchunk(g % NCH), kpp0a[:],
                                         start=True, stop=True)
                        nc.tensor.matmul(ts[:, j * 512 + 256:(j + 1) * 512],
                                         xchunk(g % NCH), kpp0b[:],
                                         start=True, stop=True)
                    else:
                        nc.tensor.matmul(ts[:, j * 512:(j + 1) * 512],
                                         xchunk(g % NCH), kpps[s - 1][:],
                                         start=True, stop=True)
                return ts, chunks

            cur = emit_scores(0)
            lag = []        # (p_tile, chunks) awaiting PV + chain, depth 2

            for t in range(NT):
                ts, chunks = cur
                p = ptp.tile([128, 1536], BF16, tag="pt")
                w = len(chunks) * 512
                if t == 0:
                    nc.scalar.activation(p[:, 0:256], ts[:, 0:256], Exp)
                    nc.scalar.activation(p[:, 256:512], ts[:, 256:512], Exp)
                else:
                    nc.scalar.activation(p[:, 0:w], ts[:, 0:w], Exp)
                if t + 1 < NT:
                    cur = emit_scores(t + 1)
                flush_due(t)
                lag.append((p, chunks))
                if len(lag) > 2:
                    emit_pv_and_chain(*lag.pop(0))

            # drain
            for item in lag:
                emit_pv_and_chain(*item)
                flush_due(NT + 10)
            flush_due(NT + 1000)

    nc.finalize()
    return nc


def _get_nc():
    if "nc" not in _cache:
        _cache["nc"] = _build()
    return _cache["nc"]


def kernel(x, Wk, bk, Wq, bq, Wv, bv, **_ignored):
    from concourse.bass_utils import run_bass_kernel_spmd

    x = np.asarray(x, dtype=np.float32)
    wp = make_wp(
        np.asarray(Wk, np.float32), np.asarray(Wq, np.float32),
        np.asarray(Wv, np.float32), np.asarray(bk, np.float32),
        np.asarray(bq, np.float32), np.asarray(bv, np.float32),
    )

    nc = _get_nc()
    in_maps = [
        {"xt": np.ascontiguousarray(x[b].T), "wp": wp} for b in range(B)
    ]
    res = run_bass_kernel_spmd(nc, in_maps, core_ids=list(range(B)))
    out = np.stack([res.results[b]["y"] for b in range(B)], axis=0)
    return out
